# revision 1
# baseline (speedup 1.0000x reference)
"""NonLocalBlock (B=4, C=256, H=W=64) Trainium2 Bass kernel.

Sharding: 8 cores = 4 batch elements x 2 query-row shards of 2048 rows.
Each core receives its batch element's x rotated along N so that its
query rows are columns [0, 2048) -- the program is identical on every
core (pure SPMD), only the data differs.

Per-core pipeline:
  A) 1x1-conv projections on the PE:
       theta[d, nq] (queries, f32r), phi[d, m] (keys, f32r),
       gT[m, d] (values, transposed chunk-major layout, fp16)
  B) attention, streamed per 512-column query block:
       S^T[m-chunk, nblk] = phi_chunk x theta   (PE, f32r logits)
       P^T = exp(S^T / sqrt(D))                 (ScalarE, -> fp16)
       y[d, nblk]  += gT_chunk.T @ P^T          (PE fp16, PSUM accum)
       r[*, nblk]  += ones.T @ P^T              (PE fp16; softmax denom,
                                                 replicated over partitions)
       rho = exp(-ln(r))                        (ScalarE; joint Exp+Ln
                                                 table set, no reloads)
       out_norm[c, nblk] = (wo.T @ y) * rho     (PE f32r out-conv, then
                                                 one DVE stt per chunk
                                                 with accum -> s1)
       s2 += sum(out_norm^2)                    (DVE stt accum)
  C) BatchNorm (training mode, batch stats): AllReduce [s1|s2] across
     all 8 cores, mean/var/rstd on-chip, then out = x + a*out + b with
     a = gamma*rstd, b = beta - a*mean.  (out_conv bias cancels in
     training-mode BN and is skipped entirely.)

float32r streams at 2 cycles/row on HW; it is kept for the
precision-critical logit path (theta/phi projections + S^T).
Post-softmax paths (P, g, ones) are fp16 -- rounding errors average
out over the 4096-key softmax sum.
"""

import math
import os

import numpy as np

import concourse.bass as bass
import concourse.mybir as mybir
import concourse.tile as tile
from concourse import bacc
from concourse.bass_utils import run_bass_kernel_spmd

# Problem constants (hardcoded per contract).
B, C, HGT, WID = 4, 256, 64, 64
N = HGT * WID            # 4096 spatial positions
D = C // 2               # 128 inner channels
P = 128                  # SBUF partitions
NCORES = 8
SPLIT = NCORES // B      # query shards per batch element
NQ = N // SPLIT          # 2048 query rows per core
CB = C // P              # 2 channel chunks
MCH = N // P             # 32 key chunks
NBLK = 512               # query block (one PSUM bank)
NB = NQ // NBLK          # 4 blocks
EPS = 1e-5
SCALE = 1.0 / math.sqrt(D)
NSAMP = float(B * N)     # BN sample count per channel

F32 = mybir.dt.float32
F32R = mybir.dt.float32r
F16 = mybir.dt.float16

AF = mybir.ActivationFunctionType
ALU = mybir.AluOpType
AX = mybir.AxisListType

_CACHED_NC = None


def _compile_with_joint_act_tables(nc):
    """Run bacc passes with Exp/Ln resolving to the joint table set.

    The default per-function chooser picks `exp_and_others` for Exp and
    `natural_log` for Ln, causing ~1.3us table reloads whenever the two
    alternate.  Emptying those two sets (preserving dict order, so the
    walrus set ids stay aligned) forces both functions onto
    `natural_log_exp_and_others`.
    """
    real = bacc.get_activation_tables

    def patched(arch):
        t = dict(real(arch))
        for k in ("exp_and_others", "natural_log"):
            if k in t:
                t[k] = type(t[k])()
        return t

    bacc.get_activation_tables = patched
    try:
        nc.compile()
    finally:
        bacc.get_activation_tables = real


def _build_nc():
    nc = bacc.Bacc("TRN2", target_bir_lowering=False, debug=False,
                   num_devices=NCORES)

    x_d = nc.dram_tensor("x", [C, N], F32R, kind="ExternalInput")
    # f32r weights: wq|wk (2*C cols) then wo (C cols)
    wp_d = nc.dram_tensor("wpack", [P, 3 * C], F32R, kind="ExternalInput")
    # fp16 value weights: wv (C cols)
    wv_d = nc.dram_tensor("wvb", [P, C], F16, kind="ExternalInput")
    # small fp32 constants: bq|bk|bv|gam|bet
    cp_d = nc.dram_tensor("cpack", [P, 1 + 1 + P + CB + CB], F32,
                          kind="ExternalInput")
    out_d = nc.dram_tensor("out", [C, NQ], F32, kind="ExternalOutput")

    with tile.TileContext(nc) as tc:
        with (
            tc.tile_pool(name="consts", bufs=1) as consts,
            tc.tile_pool(name="bigs", bufs=1) as bigs,
            tc.tile_pool(name="ptp", bufs=3) as ptp,
            tc.tile_pool(name="work", bufs=2) as work,
            tc.tile_pool(name="ps", bufs=2, space="PSUM") as ps,
            tc.tile_pool(name="dram", bufs=1, space="DRAM") as dram,
        ):
            # ---- constant / weight loads (gpsimd -> one DMASW0 sem) ----
            wpack = consts.tile([P, 3 * C], F32R)
            wvb = consts.tile([P, C], F16)
            cpack = consts.tile([P, 1 + 1 + P + CB + CB], F32)
            nc.gpsimd.dma_start(wpack[:], wp_d[:])
            nc.gpsimd.dma_start(wvb[:], wv_d[:])
            nc.gpsimd.dma_start(cpack[:], cp_d[:])
            wq = wpack[:, 0 * C:1 * C]
            wk = wpack[:, 1 * C:2 * C]
            wo = wpack[:, 2 * C:3 * C]
            bq = cpack[:, 0:1]
            bk = cpack[:, 1:2]
            bv = cpack[:, 2:2 + P]
            gam = cpack[:, 2 + P:2 + P + CB]
            bet = cpack[:, 2 + P + CB:2 + P + 2 * CB]
            ones = consts.tile([P, P], F16)
            nc.vector.memset(ones[:], 1.0)

            # ---- x load (f32r), plus fp16 cast for the value path ----
            XCH = 4
            xs = [bigs.tile([P, N], F32R, name=f"x{cb}", tag=f"x{cb}")
                  for cb in range(CB)]
            for k in range(XCH):
                ksl = slice(k * (N // XCH), (k + 1) * (N // XCH))
                for cb in range(CB):
                    nc.gpsimd.dma_start(xs[cb][:, ksl],
                                        x_d[cb * P:(cb + 1) * P, ksl])
            xb16 = [bigs.tile([P, N], F16, name=f"xb{cb}", tag=f"xb{cb}")
                    for cb in range(CB)]
            for cb in range(CB):
                for k in range(XCH):
                    ksl = slice(k * (N // XCH), (k + 1) * (N // XCH))
                    nc.vector.tensor_copy(xb16[cb][:, ksl], xs[cb][:, ksl])

            # ---- phase A: projections ----
            theta = bigs.tile([P, NQ], F32R, tag="theta")
            phi = bigs.tile([P, N], F32R, tag="phi")
            gT = bigs.tile([P, N], F16, tag="gT")  # [m%128, 128*mc + d]

            for j in range(NQ // NBLK):
                sl = slice(j * NBLK, (j + 1) * NBLK)
                pt = ps.tile([P, NBLK], F32, tag="ps_s")
                for cb in range(CB):
                    nc.tensor.matmul(
                        pt[:], wq[:, cb * P:(cb + 1) * P], xs[cb][:, sl],
                        start=(cb == 0), stop=(cb == CB - 1))
                nc.vector.tensor_scalar_add(theta[:, sl], pt[:], bq[:])
            for j in range(N // NBLK):
                sl = slice(j * NBLK, (j + 1) * NBLK)
                pt = ps.tile([P, NBLK], F32, tag="ps_s")
                for cb in range(CB):
                    nc.tensor.matmul(
                        pt[:], wk[:, cb * P:(cb + 1) * P], xs[cb][:, sl],
                        start=(cb == 0), stop=(cb == CB - 1))
                nc.vector.tensor_scalar_add(phi[:, sl], pt[:], bk[:])
            for mc in range(MCH):
                msl = slice(mc * P, (mc + 1) * P)
                pt = ps.tile([P, P], F32, tag="ps_o")
                for cb in range(CB):
                    nc.tensor.matmul(
                        pt[:], xb16[cb][:, msl], wvb[:, cb * P:(cb + 1) * P],
                        start=(cb == 0), stop=(cb == CB - 1))
                nc.vector.tensor_add(gT[:, msl], pt[:], bv[:])

            # ---- phase B: attention + out-conv + partial stats ----
            outs = [bigs.tile([P, NQ], F32, name=f"out{cb}", tag=f"out{cb}")
                    for cb in range(CB)]
            s1 = consts.tile([P, CB * NB], F32)   # per-block partial sums
            s2 = consts.tile([P, CB * NB], F32)

            for j in range(NB):
                sl = slice(j * NBLK, (j + 1) * NBLK)
                y_ps = ps.tile([P, NBLK], F32, tag="ps_y")
                r_ps = ps.tile([P, NBLK], F32, tag="ps_r")
                for mc in range(MCH):
                    msl = slice(mc * P, (mc + 1) * P)
                    s_ps = ps.tile([P, NBLK], F32, tag="ps_s")
                    nc.tensor.matmul(s_ps[:], phi[:, msl], theta[:, sl],
                                     start=True, stop=True)
                    pT = ptp.tile([P, NBLK], F16, tag="pT")
                    nc.scalar.activation(pT[:], s_ps[:], AF.Exp, scale=SCALE)
                    nc.tensor.matmul(y_ps[:], gT[:, msl], pT[:],
                                     start=(mc == 0), stop=(mc == MCH - 1))
                    nc.tensor.matmul(r_ps[:], ones[:], pT[:],
                                     start=(mc == 0), stop=(mc == MCH - 1))
                # rho = 1/r via exp(-ln(r)) on ScalarE (joint table set)
                lnr = work.tile([P, NBLK], F32, tag="lnr")
                nc.scalar.activation(lnr[:], r_ps[:], AF.Ln)
                rho = work.tile([P, NBLK], F32, tag="rho")
                nc.scalar.activation(rho[:], lnr[:], AF.Exp, scale=-1.0)
                # unnormalized y to SBUF (f32r) for the out-conv
                ysb = work.tile([P, NBLK], F32R, tag="ysb")
                nc.vector.tensor_copy(ysb[:], y_ps[:])
                for cb in range(CB):
                    o_ps = ps.tile([P, NBLK], F32, tag="ps_o")
                    nc.tensor.matmul(o_ps[:], wo[:, cb * P:(cb + 1) * P],
                                     ysb[:], start=True, stop=True)
                    col = slice(cb * NB + j, cb * NB + j + 1)
                    # normalize + copy out + sum(out) in one DVE op
                    nc.vector.scalar_tensor_tensor(
                        out=outs[cb][:, sl], in0=o_ps[:], scalar=1.0,
                        in1=rho[:], op0=ALU.mult, op1=ALU.mult,
                        accum_out=s1[:, col])
                    sq = work.tile([P, NBLK], F32, tag="sq")
                    nc.vector.scalar_tensor_tensor(
                        out=sq[:], in0=outs[cb][:, sl], scalar=1.0,
                        in1=outs[cb][:, sl], op0=ALU.mult, op1=ALU.mult,
                        accum_out=s2[:, col])

            # ---- phase C: BN stats allreduce + apply + residual ----
            stats = consts.tile([P, 2 * CB], F32)
            for cb in range(CB):
                nc.vector.tensor_reduce(
                    stats[:, cb:cb + 1], s1[:, cb * NB:(cb + 1) * NB],
                    axis=AX.X, op=ALU.add)
                nc.vector.tensor_reduce(
                    stats[:, CB + cb:CB + cb + 1], s2[:, cb * NB:(cb + 1) * NB],
                    axis=AX.X, op=ALU.add)

            cc_in = dram.tile([P, 2 * CB], F32)
            cc_out = dram.tile([P, 2 * CB], F32)
            nc.sync.dma_start(cc_in[:], stats[:])
            nc.gpsimd.collective_compute(
                "AllReduce", ALU.add,
                replica_groups=[list(range(NCORES))],
                ins=[cc_in[:].opt()], outs=[cc_out[:].opt()])
            gstats = consts.tile([P, 2 * CB], F32)
            nc.sync.dma_start(gstats[:], cc_out[:])

            mean = consts.tile([P, CB], F32)
            var = consts.tile([P, CB], F32)
            tmp = consts.tile([P, CB], F32)
            rstd = consts.tile([P, CB], F32)
            a_sc = consts.tile([P, CB], F32)
            b_sc = consts.tile([P, CB], F32)
            nc.vector.tensor_scalar_mul(mean[:], gstats[:, 0:CB], 1.0 / NSAMP)
            nc.vector.tensor_mul(tmp[:], mean[:], mean[:])
            # var = s2/NSAMP - mean^2
            nc.vector.scalar_tensor_tensor(
                out=var[:], in0=gstats[:, CB:2 * CB], scalar=1.0 / NSAMP,
                in1=tmp[:], op0=ALU.mult, op1=ALU.subtract)
            # rstd = exp(-0.5 * ln(var + eps))
            eps_t = consts.tile([P, 1], F32)
            nc.vector.memset(eps_t[:], EPS)
            nc.scalar.activation(tmp[:], var[:], AF.Ln, bias=eps_t[:])
            nc.scalar.activation(rstd[:], tmp[:], AF.Exp, scale=-0.5)
            nc.vector.tensor_mul(a_sc[:], gam[:], rstd[:])
            nc.vector.tensor_mul(tmp[:], a_sc[:], mean[:])
            nc.vector.tensor_sub(b_sc[:], bet[:], tmp[:])

            for cb in range(CB):
                xb = work.tile([P, NQ], F32, tag="xb")
                nc.vector.tensor_scalar_add(xb[:], xs[cb][:, 0:NQ],
                                            b_sc[:, cb:cb + 1])
                for j in range(NB):
                    sl = slice(j * NBLK, (j + 1) * NBLK)
                    f = work.tile([P, NBLK], F32, tag="f")
                    nc.vector.scalar_tensor_tensor(
                        out=f[:], in0=outs[cb][:, sl], scalar=a_sc[:, cb:cb + 1],
                        in1=xb[:, sl], op0=ALU.mult, op1=ALU.add)
                    nc.sync.dma_start(out_d[cb * P:(cb + 1) * P, sl], f[:])

    _compile_with_joint_act_tables(nc)
    return nc


def _get_nc():
    global _CACHED_NC
    if _CACHED_NC is None:
        _CACHED_NC = _build_nc()
    return _CACHED_NC


def _in_maps(inputs):
    x = np.ascontiguousarray(np.asarray(inputs["x"], np.float32)).reshape(B, C, N)
    tw = np.asarray(inputs["theta_w"], np.float32)
    pw = np.asarray(inputs["phi_w"], np.float32)
    gw = np.asarray(inputs["g_w"], np.float32)
    ow = np.asarray(inputs["out_w"], np.float32)

    def pack_ct(w):  # [D, C] -> [128, C] chunk-major transposed
        wt = np.ascontiguousarray(w.T)            # [C, D]
        return np.concatenate([wt[cb * P:(cb + 1) * P, :] for cb in range(CB)],
                              axis=1)             # [P, CB*D]

    wpack = np.concatenate(
        [pack_ct(tw), pack_ct(pw),
         np.ascontiguousarray(ow.T)], axis=1)     # [128, 3*256]
    wvb = pack_ct(gw).astype(np.float16)
    bq = np.asarray(inputs["theta_b"], np.float32).reshape(P, 1)
    bk = np.asarray(inputs["phi_b"], np.float32).reshape(P, 1)
    bv = np.broadcast_to(np.asarray(inputs["g_b"], np.float32)[None, :], (P, P))
    gam = np.asarray(inputs["gamma"], np.float32).reshape(CB, P).T
    bet = np.asarray(inputs["beta"], np.float32).reshape(CB, P).T
    cpack = np.ascontiguousarray(
        np.concatenate([bq, bk, bv, gam, bet], axis=1))  # [128, 134]

    maps = []
    for core in range(NCORES):
        b, h = divmod(core, SPLIT)
        n0 = h * NQ
        xr = x[b] if n0 == 0 else np.ascontiguousarray(
            np.concatenate([x[b][:, n0:], x[b][:, :n0]], axis=1))
        maps.append({"x": xr, "wpack": wpack, "wvb": wvb, "cpack": cpack})
    return maps


def _run(inputs, trace=False, **kw):
    nc = _get_nc()
    maps = _in_maps(inputs)
    r = run_bass_kernel_spmd(nc, maps, list(range(NCORES)), trace=trace, **kw)
    out = np.empty((B, C, N), np.float32)
    for core in range(NCORES):
        b, h = divmod(core, SPLIT)
        out[b][:, h * NQ:(h + 1) * NQ] = r.results[core]["out"]
    return out.reshape(B, C, HGT, WID), r


def kernel(**inputs):
    out, _ = _run(inputs, trace=False)
    return out



# revision 4
# speedup vs baseline: 1.2678x; 1.2678x over previous
"""NonLocalBlock (B=4, C=256, H=W=64) Trainium2 Bass kernel.

Sharding: 8 cores = 4 batch elements x 2 query-row shards of 2048 rows.
Each core receives its batch element's x rotated along N so that its
query rows are columns [0, 2048) -- the program is identical on every
core (pure SPMD), only the data differs.

v2 design notes (all-fp16 data path, engine-balanced):
  * x arrives pre-cast to fp16 on the host (2MB instead of 4MB per
    core); the residual add also uses fp16 x (error ~1e-4 << 2e-2 gate).
  * g-projection bias and out-conv bias are skipped entirely: softmax
    columns sum to 1, so g_b adds a constant per channel to y which the
    out-conv turns into a per-channel constant -- exactly cancelled by
    training-mode BN (verified numerically to 4e-15).
  * Attention runs in two 1024-query "pairs".  Per key-chunk mc:
    two S matmuls (one LDWEIGHTS) -> [128,1024] 2-bank PSUM tile,
    one fused exp over both banks (ScalarE, ~1.07us), two y matmuls
    accumulating in a persistent 2-bank PSUM tile.  S matmuls are
    emitted one iteration ahead of y so the in-order PE queue never
    waits on ScalarE.  PSUM: 3x2 (S) + 2 (y) = 8 banks.
  * softmax denominator: DVE running-sum of pT chunks (fp16 4x mode,
    ~330ns each) + a single ones-matmul per pair, instead of 32
    ones-matmuls per pair on the PE.
  * rho = 1/r via DVE reciprocal (keeps ScalarE exp-only in the loop);
    BN rstd still uses the joint Exp/Ln table (no table reloads).
  * BN stats AllReduce (2KB) into a Shared-addr-space DRAM tensor.
  * Output DMA'd as fp16 and upcast on the host.
"""

import math

import numpy as np

import concourse.bass as bass
import concourse.mybir as mybir
import concourse.tile as tile
from concourse import bacc
from concourse.bass_utils import run_bass_kernel_spmd

# Problem constants (hardcoded per contract).
B, C, HGT, WID = 4, 256, 64, 64
N = HGT * WID            # 4096 spatial positions
D = C // 2               # 128 inner channels
P = 128                  # SBUF partitions
NCORES = 8
SPLIT = NCORES // B      # query shards per batch element
NQ = N // SPLIT          # 2048 query rows per core
CB = C // P              # 2 channel chunks
MCH = N // P             # 32 key chunks
QPAIR = 1024             # query block processed per inner loop (2 PSUM banks)
NPAIR = NQ // QPAIR      # 2 pairs
EPS = 1e-5
SCALE = 1.0 / math.sqrt(D)
NSAMP = float(B * N)     # BN sample count per channel

F32 = mybir.dt.float32
F16 = mybir.dt.float16

AF = mybir.ActivationFunctionType
ALU = mybir.AluOpType
AX = mybir.AxisListType

_CACHED_NC = None


def _compile_with_joint_act_tables(nc):
    """Run bacc passes with Exp/Ln resolving to the joint table set.

    The default per-function chooser picks `exp_and_others` for Exp and
    `natural_log` for Ln, causing ~1.3us table reloads whenever the two
    alternate.  Emptying those two sets (preserving dict order, so the
    walrus set ids stay aligned) forces both functions onto
    `natural_log_exp_and_others`.
    """
    real = bacc.get_activation_tables

    def patched(arch):
        t = dict(real(arch))
        for k in ("exp_and_others", "natural_log"):
            if k in t:
                t[k] = type(t[k])()
        return t

    bacc.get_activation_tables = patched
    try:
        nc.compile()
    finally:
        bacc.get_activation_tables = real


def _build_nc():
    nc = bacc.Bacc("TRN2", target_bir_lowering=False, debug=False,
                   num_devices=NCORES)

    # fp16 x, chunk-major: cols [cb*N, (cb+1)*N) hold channel chunk cb.
    xh_d = nc.dram_tensor("xh", [P, CB * N], F16, kind="ExternalInput")
    # fp16 weights: wq|wk (2*C cols, chunk-major transposed) then wo (C cols)
    wp_d = nc.dram_tensor("wpack", [P, 3 * C], F16, kind="ExternalInput")
    # fp16 value weights (C cols, chunk-major transposed)
    wv_d = nc.dram_tensor("wvb", [P, C], F16, kind="ExternalInput")
    # small fp32 constants: bq|bk|gam|bet
    cp_d = nc.dram_tensor("cpack", [P, 1 + 1 + CB + CB], F32,
                          kind="ExternalInput")
    out_d = nc.dram_tensor("out", [C, NQ], F16, kind="ExternalOutput")

    with tile.TileContext(nc) as tc:
        with (
            tc.tile_pool(name="consts", bufs=1) as consts,
            tc.tile_pool(name="bigs", bufs=1) as bigs,
            tc.tile_pool(name="ptp", bufs=4) as ptp,
            tc.tile_pool(name="work", bufs=2) as work,
            tc.tile_pool(name="ps", bufs=1, space="PSUM") as ps,
            tc.tile_pool(name="dram", bufs=1, space="DRAM") as dram,
        ):
            # ---- weight / constant loads ----
            wpack = consts.tile([P, 3 * C], F16)
            wvb = consts.tile([P, C], F16)
            cpack = consts.tile([P, 1 + 1 + CB + CB], F32)
            nc.sync.dma_start(wpack[:], wp_d[:])
            nc.sync.dma_start(cpack[:], cp_d[:])
            nc.gpsimd.dma_start(wvb[:], wv_d[:])
            wq = wpack[:, 0 * C:1 * C]
            wk = wpack[:, 1 * C:2 * C]
            wo = wpack[:, 2 * C:3 * C]
            bq = cpack[:, 0:1]
            bk = cpack[:, 1:2]
            gam = cpack[:, 2:2 + CB]
            bet = cpack[:, 2 + CB:2 + 2 * CB]
            ones = consts.tile([P, P], F16)
            nc.vector.memset(ones[:], 1.0)

            # ---- x load: 8 chunks of [128,1024], cb-interleaved so the
            # projections can start after the first two chunks ----
            xh = bigs.tile([P, CB * N], F16, tag="xh")
            XCH = 4
            for k in range(XCH):
                ksl = slice(k * (N // XCH), (k + 1) * (N // XCH))
                for cb in range(CB):
                    csl = slice(cb * N + ksl.start, cb * N + ksl.stop)
                    eng = nc.sync if (k + cb) % 2 == 0 else nc.gpsimd
                    eng.dma_start(xh[:, csl], xh_d[:, csl])

            def xch(cb, sl):  # x channel-chunk cb, position slice sl
                return xh[:, cb * N + sl.start:cb * N + sl.stop]

            # ---- phase A: projections (no g bias: cancels in BN) ----
            th16 = bigs.tile([P, NQ], F16, tag="th16")
            ph16 = bigs.tile([P, N], F16, tag="ph16")
            gT16 = bigs.tile([P, N], F16, tag="gT16")  # [m%128, 128*mc + d]

            for j in range(NQ // QPAIR):
                sl = slice(j * QPAIR, (j + 1) * QPAIR)
                pt = ps.tile([P, QPAIR], F32, tag="ps_s", bufs=3)
                for cb in range(CB):
                    for h in range(2):
                        hs = slice(h * 512, (h + 1) * 512)
                        xsl = slice(sl.start + hs.start, sl.start + hs.stop)
                        nc.tensor.matmul(
                            pt[:, hs], wq[:, cb * P:(cb + 1) * P],
                            xch(cb, xsl),
                            start=(cb == 0), stop=(cb == CB - 1))
                nc.vector.tensor_scalar_add(th16[:, sl], pt[:], bq[:])
            for j in range(N // QPAIR):
                sl = slice(j * QPAIR, (j + 1) * QPAIR)
                pt = ps.tile([P, QPAIR], F32, tag="ps_s", bufs=3)
                for cb in range(CB):
                    for h in range(2):
                        hs = slice(h * 512, (h + 1) * 512)
                        xsl = slice(sl.start + hs.start, sl.start + hs.stop)
                        nc.tensor.matmul(
                            pt[:, hs], wk[:, cb * P:(cb + 1) * P],
                            xch(cb, xsl),
                            start=(cb == 0), stop=(cb == CB - 1))
                nc.vector.tensor_scalar_add(ph16[:, sl], pt[:], bk[:])
            for mc in range(MCH):
                msl = slice(mc * P, (mc + 1) * P)
                pt = ps.tile([P, QPAIR], F32, tag="ps_s", bufs=3)
                for cb in range(CB):
                    nc.tensor.matmul(
                        pt[:, 0:P], xch(cb, msl), wvb[:, cb * P:(cb + 1) * P],
                        start=(cb == 0), stop=(cb == CB - 1))
                nc.vector.tensor_copy(gT16[:, msl], pt[:, 0:P])

            # ---- phase B: attention (software-pipelined S ahead of y) ----
            o16 = bigs.tile([P, CB * NQ], F16, tag="o16")
            stats = consts.tile([P, 4 * NPAIR], F32)  # s1 x4 then s2 x4? see cols

            def s_matmuls(pair, mc, s_ps):
                q0 = pair * QPAIR
                msl = slice(mc * P, (mc + 1) * P)
                for h in range(2):
                    hs = slice(h * 512, (h + 1) * 512)
                    nc.tensor.matmul(
                        s_ps[:, hs], ph16[:, msl],
                        th16[:, q0 + hs.start:q0 + hs.stop],
                        start=True, stop=True)

            for pair in range(NPAIR):
                y_ps = ps.tile([P, QPAIR], F32, tag="ps_y", bufs=1)
                acc = work.tile([P, QPAIR], F16, tag="acc")
                s_tiles = [ps.tile([P, QPAIR], F32, tag="ps_s", bufs=3,
                                   name=f"s{pair}_{i}")
                           for i in range(MCH)]
                s_matmuls(pair, 0, s_tiles[0])
                for mc in range(MCH):
                    if mc + 1 < MCH:
                        s_matmuls(pair, mc + 1, s_tiles[mc + 1])
                    pT = ptp.tile([P, QPAIR], F16, tag="pT")
                    nc.scalar.activation(pT[:], s_tiles[mc][:], AF.Exp,
                                         scale=SCALE)
                    msl = slice(mc * P, (mc + 1) * P)
                    for h in range(2):
                        hs = slice(h * 512, (h + 1) * 512)
                        nc.tensor.matmul(y_ps[:, hs], gT16[:, msl], pT[:, hs],
                                         start=(mc == 0), stop=(mc == MCH - 1))
                    if mc == 0:
                        nc.vector.tensor_copy(acc[:], pT[:])
                    else:
                        nc.vector.tensor_add(acc[:], acc[:], pT[:])

                # pair tail: denominator, normalize, out-conv, partial stats
                r_ps = ps.tile([P, QPAIR], F32, tag="ps_s", bufs=3)
                for h in range(2):
                    hs = slice(h * 512, (h + 1) * 512)
                    nc.tensor.matmul(r_ps[:, hs], ones[:], acc[:, hs],
                                     start=True, stop=True)
                rho = work.tile([P, QPAIR], F32, tag="rho")
                nc.vector.reciprocal(rho[:], r_ps[:])
                ysb = work.tile([P, QPAIR], F16, tag="ysb")
                nc.vector.tensor_mul(ysb[:], y_ps[:], rho[:])
                for cb in range(CB):
                    o_ps = ps.tile([P, QPAIR], F32, tag="ps_s", bufs=3)
                    for h in range(2):
                        hs = slice(h * 512, (h + 1) * 512)
                        nc.tensor.matmul(o_ps[:, hs], wo[:, cb * P:(cb + 1) * P],
                                         ysb[:, hs], start=True, stop=True)
                    osl = slice(cb * NQ + pair * QPAIR,
                                cb * NQ + (pair + 1) * QPAIR)
                    col = pair * CB + cb
                    nc.vector.tensor_scalar(
                        out=o16[:, osl], in0=o_ps[:], scalar1=1.0, scalar2=None,
                        op0=ALU.mult, op1=ALU.add,
                        accum_out=stats[:, col:col + 1])
                    sq = work.tile([P, QPAIR], F16, tag="sq")
                    nc.vector.scalar_tensor_tensor(
                        out=sq[:], in0=o16[:, osl], scalar=1.0,
                        in1=o16[:, osl], op0=ALU.mult, op1=ALU.mult,
                        accum_out=stats[:, 4 + col:4 + col + 1])

            # ---- phase C: BN stats allreduce + apply + residual ----
            # stats cols: [p0cb0 p0cb1 p1cb0 p1cb1] s1, then same for s2.
            cstat = consts.tile([P, 2 * CB], F32)
            nc.vector.tensor_add(cstat[:, 0:CB], stats[:, 0:CB],
                                 stats[:, CB:2 * CB])
            nc.vector.tensor_add(cstat[:, CB:2 * CB], stats[:, 4:4 + CB],
                                 stats[:, 4 + CB:4 + 2 * CB])

            cc_in = dram.tile([P, 2 * CB], F32)
            cc_out = dram.tile([P, 2 * CB], F32, addr_space="Shared")
            nc.sync.dma_start(cc_in[:], cstat[:])
            nc.gpsimd.collective_compute(
                "AllReduce", ALU.add,
                replica_groups=[list(range(NCORES))],
                ins=[cc_in[:].opt()], outs=[cc_out[:].opt()])
            gstats = consts.tile([P, 2 * CB], F32)
            nc.sync.dma_start(gstats[:], cc_out[:])

            mean = consts.tile([P, CB], F32)
            var = consts.tile([P, CB], F32)
            tmp = consts.tile([P, CB], F32)
            rstd = consts.tile([P, CB], F32)
            a_sc = consts.tile([P, CB], F32)
            b_sc = consts.tile([P, CB], F32)
            nc.vector.tensor_scalar_mul(mean[:], gstats[:, 0:CB], 1.0 / NSAMP)
            nc.vector.tensor_mul(tmp[:], mean[:], mean[:])
            # var = s2/NSAMP - mean^2
            nc.vector.scalar_tensor_tensor(
                out=var[:], in0=gstats[:, CB:2 * CB], scalar=1.0 / NSAMP,
                in1=tmp[:], op0=ALU.mult, op1=ALU.subtract)
            # rstd = exp(-0.5 * ln(var + eps)) (joint Exp/Ln table set)
            eps_t = consts.tile([P, 1], F32)
            nc.vector.memset(eps_t[:], EPS)
            nc.scalar.activation(tmp[:], var[:], AF.Ln, bias=eps_t[:])
            nc.scalar.activation(rstd[:], tmp[:], AF.Exp, scale=-0.5)
            nc.vector.tensor_mul(a_sc[:], gam[:], rstd[:])
            nc.vector.tensor_mul(tmp[:], a_sc[:], mean[:])
            nc.vector.tensor_sub(b_sc[:], bet[:], tmp[:])

            for cb in range(CB):
                xb = work.tile([P, NQ], F16, tag="xb")
                nc.vector.tensor_scalar_add(xb[:], xch(cb, slice(0, NQ)),
                                            b_sc[:, cb:cb + 1])
                f = work.tile([P, NQ], F16, tag="f")
                nc.vector.scalar_tensor_tensor(
                    out=f[:], in0=o16[:, cb * NQ:(cb + 1) * NQ],
                    scalar=a_sc[:, cb:cb + 1],
                    in1=xb[:], op0=ALU.mult, op1=ALU.add)
                eng = nc.sync if cb == 0 else nc.gpsimd
                eng.dma_start(out_d[cb * P:(cb + 1) * P, :], f[:])

    _compile_with_joint_act_tables(nc)
    return nc


def _get_nc():
    global _CACHED_NC
    if _CACHED_NC is None:
        _CACHED_NC = _build_nc()
    return _CACHED_NC


def _in_maps(inputs):
    x = np.ascontiguousarray(np.asarray(inputs["x"], np.float32)).reshape(B, C, N)
    tw = np.asarray(inputs["theta_w"], np.float32)
    pw = np.asarray(inputs["phi_w"], np.float32)
    gw = np.asarray(inputs["g_w"], np.float32)
    ow = np.asarray(inputs["out_w"], np.float32)

    def pack_ct(w):  # [D, C] -> [128, C] chunk-major transposed
        wt = np.ascontiguousarray(w.T)            # [C, D]
        return np.concatenate([wt[cb * P:(cb + 1) * P, :] for cb in range(CB)],
                              axis=1)             # [P, CB*D]

    wpack = np.concatenate(
        [pack_ct(tw), pack_ct(pw),
         np.ascontiguousarray(ow.T)], axis=1).astype(np.float16)  # [128, 768]
    wvb = pack_ct(gw).astype(np.float16)
    bq = np.asarray(inputs["theta_b"], np.float32).reshape(P, 1)
    bk = np.asarray(inputs["phi_b"], np.float32).reshape(P, 1)
    gam = np.asarray(inputs["gamma"], np.float32).reshape(CB, P).T
    bet = np.asarray(inputs["beta"], np.float32).reshape(CB, P).T
    cpack = np.ascontiguousarray(
        np.concatenate([bq, bk, gam, bet], axis=1))  # [128, 6]

    maps = []
    for core in range(NCORES):
        b, h = divmod(core, SPLIT)
        n0 = h * NQ
        xr = x[b] if n0 == 0 else np.concatenate(
            [x[b][:, n0:], x[b][:, :n0]], axis=1)
        # [C, N] -> [128, CB*N] chunk-major fp16
        xhp = np.ascontiguousarray(
            np.concatenate([xr[cb * P:(cb + 1) * P, :] for cb in range(CB)],
                           axis=1)).astype(np.float16)
        maps.append({"xh": xhp, "wpack": wpack, "wvb": wvb, "cpack": cpack})
    return maps


def _run(inputs, trace=False, **kw):
    nc = _get_nc()
    maps = _in_maps(inputs)
    r = run_bass_kernel_spmd(nc, maps, list(range(NCORES)), trace=trace, **kw)
    out = np.empty((B, C, N), np.float32)
    for core in range(NCORES):
        b, h = divmod(core, SPLIT)
        out[b][:, h * NQ:(h + 1) * NQ] = r.results[core]["out"].astype(np.float32)
    return out.reshape(B, C, HGT, WID), r


def kernel(**inputs):
    out, _ = _run(inputs, trace=False)
    return out
